# revision 3
# baseline (speedup 1.0000x reference)
"""GQA decode attention (b=32, T=4096, 64 q-heads / 8 kv-heads) on 8 trn2 cores.

Tensor-parallel over heads: core i owns kv-head i (q-heads 8i..8i+7),
wqkv block i, KV-cache slice i, wo input-rows 1024i..1024(i+1); AllReduce
after wo.

Host-side layout prep (numerically equivalent, layout only):
  - RoPE is linear in q/k for a fixed position, so it is folded into the
    wqkv weight columns (q also absorbs the 1/sqrt(128) score scale).
  - K slice pre-transposed to [b, d, t] so score matmuls contract d on
    partitions; V packed partition-major; wo slice pre-transposed.
"""

import math
import sys

import numpy as np

sys.path.insert(0, "/opt/trn_rl_repo")

B = 32          # batch
D = 8192        # model dim
HD = 128        # head dim
H = 8           # q-heads per core
NKV = 8         # kv heads (= cores)
T = 4096        # kv length
NT = T // 128   # t-tiles
KD = D // 128   # k-tiles over model dim
BLK = 1280      # wqkv block per kv head (8*128 q | 128 k | 128 v)

_CACHE: dict = {}


def _build():
    from contextlib import ExitStack

    import concourse.tile as tile
    from concourse import bacc, mybir
    from concourse.masks import make_identity

    f32 = mybir.dt.float32
    nc = bacc.Bacc("TRN2", target_bir_lowering=False, debug=False, num_devices=8)

    xT = nc.dram_tensor("xT", [128, KD, B], f32, kind="ExternalInput")
    wq = nc.dram_tensor("wq", [D, BLK], f32, kind="ExternalInput")
    kT = nc.dram_tensor("kT", [B, 128, T], f32, kind="ExternalInput")
    vv = nc.dram_tensor("vv", [B, 128, NT, HD], f32, kind="ExternalInput")
    woT = nc.dram_tensor("woT", [H * HD, D], f32, kind="ExternalInput")
    out_ext = nc.dram_tensor("out", [B, D], f32, kind="ExternalOutput")

    ExpF = mybir.ActivationFunctionType.Exp

    with tile.TileContext(nc) as tc, ExitStack() as ctx:
        cst = ctx.enter_context(tc.tile_pool(name="const", bufs=1))
        ident = cst.tile([128, 128], f32)
        make_identity(nc, ident[:])
        ones = cst.tile([128, 1], f32)
        nc.vector.memset(ones[:], 1.0)

        qT_sb = cst.tile([128, H, B], f32)      # q^T  [d, h, b]
        knT_sb = cst.tile([128, B], f32)        # k_new^T [d, b]
        vn_sb = cst.tile([B, HD], f32)          # v_new [b, d]
        attT_sb = cst.tile([128, H, B], f32)    # att^T [d, h, b]

        # ---------------- phase 1: fused qkv projection ----------------
        with (
            tc.tile_pool(name="w", bufs=3) as wpool,
            tc.tile_pool(name="xt", bufs=1) as xpool,
            tc.tile_pool(name="qps", bufs=1, space="PSUM") as qps,
            tc.tile_pool(name="m1", bufs=1) as m1,
            tc.tile_pool(name="tps", bufs=1, space="PSUM") as tps,
        ):
            xt = xpool.tile([128, KD, B], f32)
            nc.sync.dma_start(xt[:], xT[:])
            ps_q1 = qps.tile([B, 512], f32)
            ps_q2 = qps.tile([B, 512], f32)
            ps_kv = qps.tile([B, 256], f32)
            for k in range(KD):
                wt = wpool.tile([128, BLK], f32)
                nc.sync.dma_start(wt[:], wq[k * 128:(k + 1) * 128, :])
                lhs = xt[:, k, :]
                st, sp = k == 0, k == KD - 1
                nc.tensor.matmul(ps_q1[:], lhs, wt[:, 0:512], start=st, stop=sp)
                nc.tensor.matmul(ps_q2[:], lhs, wt[:, 512:1024], start=st, stop=sp)
                nc.tensor.matmul(ps_kv[:], lhs, wt[:, 1024:1280], start=st, stop=sp)

            q_sb = m1.tile([B, 1024], f32)
            nc.vector.tensor_copy(q_sb[:, 0:512], ps_q1[:])
            nc.vector.tensor_copy(q_sb[:, 512:1024], ps_q2[:])
            kv_sb = m1.tile([B, 256], f32)
            nc.vector.tensor_copy(kv_sb[:], ps_kv[:])
            nc.vector.tensor_copy(vn_sb[:], kv_sb[:, 128:256])

            t_ps = tps.tile([128, H, B], f32)
            for h in range(H):
                nc.tensor.transpose(
                    t_ps[:, h, :], q_sb[:, h * 128:(h + 1) * 128], ident[0:B, 0:B]
                )
            nc.vector.tensor_copy(qT_sb[:], t_ps[:])
            t2_ps = tps.tile([128, B], f32)
            nc.tensor.transpose(t2_ps[:], kv_sb[:, 0:128], ident[0:B, 0:B])
            nc.vector.tensor_copy(knT_sb[:], t2_ps[:])

        # ---------------- phase 2: attention over batches ----------------
        with (
            tc.tile_pool(name="kt", bufs=2) as ktp,
            tc.tile_pool(name="vt", bufs=2) as vtp,
            tc.tile_pool(name="pr", bufs=2) as prp,
            tc.tile_pool(name="scps", bufs=2, space="PSUM") as scp,
            tc.tile_pool(name="ovps", bufs=2, space="PSUM") as ovp,
            tc.tile_pool(name="dnps", bufs=2, space="PSUM") as dnp,
            tc.tile_pool(name="atps", bufs=2, space="PSUM") as atp,
            tc.tile_pool(name="att", bufs=2) as attp,
        ):
            for b in range(B):
                kt_t = ktp.tile([128, T], f32)
                nc.sync.dma_start(kt_t[:], kT[b])
                # overwrite column start_pos with the new (rope'd) k
                nc.vector.tensor_copy(kt_t[:, T - 1:T], knT_sb[:, b:b + 1])

                vt_t = vtp.tile([128, NT, HD], f32)
                nc.sync.dma_start(vt_t[:], vv[b])
                # overwrite row start_pos (= tile NT-1, partition 127) with new v
                nc.sync.dma_start(vt_t[127:128, NT - 1, :], vn_sb[b:b + 1, :])

                sc = scp.tile([128, NT, H], f32)
                for j in range(NT):
                    nc.tensor.matmul(
                        sc[:, j, :], kt_t[:, j * 128:(j + 1) * 128], qT_sb[:, :, b],
                        start=True, stop=True,
                    )
                pr = prp.tile([128, NT, H], f32)
                nc.scalar.activation(pr[:], sc[:], ExpF)

                ov = ovp.tile([H, HD], f32)
                dn = dnp.tile([H, 1], f32)
                for j in range(NT):
                    st, sp = j == 0, j == NT - 1
                    nc.tensor.matmul(ov[:], pr[:, j, :], vt_t[:, j, :], start=st, stop=sp)
                    nc.tensor.matmul(dn[:], pr[:, j, :], ones[:], start=st, stop=sp)

                rec = attp.tile([H, 1], f32)
                nc.vector.reciprocal(rec[:], dn[:])
                att_b = attp.tile([H, HD], f32)
                nc.vector.tensor_scalar_mul(att_b[:], ov[:], rec[:])
                at_ps = atp.tile([128, H], f32)
                nc.tensor.transpose(at_ps[:], att_b[:], ident[0:H, 0:H])
                nc.vector.tensor_copy(attT_sb[:, :, b], at_ps[:])

        # ---------------- phase 3: wo row-parallel + AllReduce ----------------
        with (
            tc.tile_pool(name="wo", bufs=3) as wop,
            tc.tile_pool(name="wops", bufs=1, space="PSUM") as wops,
            tc.tile_pool(name="ob", bufs=4) as obp,
            tc.tile_pool(name="dram", bufs=1, space="DRAM") as dram,
        ):
            cc_in = dram.tile([B, D], f32)
            cc_out = dram.tile([B, D], f32)
            for half in range(2):
                ps_list = [
                    wops.tile([B, 512], f32, name=f"wops{n}", tag=f"wops{n}")
                    for n in range(8)
                ]
                for k in range(H):
                    wt = wop.tile([128, 4096], f32)
                    nc.sync.dma_start(
                        wt[:], woT[k * 128:(k + 1) * 128, half * 4096:(half + 1) * 4096]
                    )
                    for n in range(8):
                        nc.tensor.matmul(
                            ps_list[n][:], attT_sb[:, k, :], wt[:, n * 512:(n + 1) * 512],
                            start=(k == 0), stop=(k == H - 1),
                        )
                for n in range(8):
                    ob = obp.tile([B, 512], f32)
                    nc.vector.tensor_copy(ob[:], ps_list[n][:])
                    base = half * 4096 + n * 512
                    nc.sync.dma_start(cc_in[:, base:base + 512], ob[:])

            nc.gpsimd.collective_compute(
                "AllReduce",
                mybir.AluOpType.add,
                replica_groups=[list(range(8))],
                ins=[cc_in.opt()],
                outs=[cc_out.opt()],
            )
            nc.sync.dma_start(out_ext[:], cc_out[:])

    nc.compile()
    return nc


def _prep_inputs(x, cache_k, cache_v, wqkv_w, wo_w, freqs_cos, freqs_sin):
    cos = np.asarray(freqs_cos, np.float32).reshape(-1)[:64]
    sin = np.asarray(freqs_sin, np.float32).reshape(-1)[:64]
    x = np.asarray(x, np.float32).reshape(B, D)
    # x^T packed tile-major: xT[p, k, b] = x[b, 128k+p]
    xT = np.ascontiguousarray(x.reshape(B, KD, 128).transpose(2, 1, 0))

    wqkv_w = np.asarray(wqkv_w, np.float32)
    scale = 1.0 / math.sqrt(HD)
    in_maps = []
    for c in range(8):
        W = wqkv_w[:, c * BLK:(c + 1) * BLK].copy()
        q = W[:, :1024].reshape(D, H, 64, 2)
        q0 = q[..., 0].copy()
        q1 = q[..., 1].copy()
        q[..., 0] = (q0 * cos - q1 * sin) * scale
        q[..., 1] = (q0 * sin + q1 * cos) * scale
        k = W[:, 1024:1152].reshape(D, 64, 2)
        k0 = k[..., 0].copy()
        k1 = k[..., 1].copy()
        k[..., 0] = k0 * cos - k1 * sin
        k[..., 1] = k0 * sin + k1 * cos

        kTc = np.ascontiguousarray(
            np.asarray(cache_k[:, :, c, :], np.float32).transpose(0, 2, 1)
        )  # [B, 128, T]
        vc = np.ascontiguousarray(
            np.asarray(cache_v[:, :, c, :], np.float32)
            .reshape(B, NT, 128, HD)
            .transpose(0, 2, 1, 3)
        )  # [B, 128, NT, HD]  (partition-major)
        woTc = np.ascontiguousarray(
            np.asarray(wo_w[:, c * 1024:(c + 1) * 1024], np.float32).T
        )  # [1024, D]
        in_maps.append({"xT": xT, "wq": W, "kT": kTc, "vv": vc, "woT": woTc})
    return in_maps


def kernel(x, cache_k, cache_v, wqkv_w, wo_w, freqs_cos, freqs_sin, mask,
           start_pos, _want_trace=False, **_unused):
    from concourse.bass_utils import run_bass_kernel_spmd

    sp = int(np.asarray(start_pos))
    assert sp == T - 1, f"kernel compiled for start_pos={T - 1}, got {sp}"

    if "nc" not in _CACHE:
        _CACHE["nc"] = _build()
    nc = _CACHE["nc"]

    in_maps = _prep_inputs(x, cache_k, cache_v, wqkv_w, wo_w, freqs_cos, freqs_sin)
    res = run_bass_kernel_spmd(nc, in_maps, list(range(8)), trace=_want_trace)
    out = res.results[0]["out"].reshape(B, 1, D).astype(np.float32)
    if _want_trace:
        _CACHE["last_result"] = res
    return out


# revision 5
# speedup vs baseline: 1.8883x; 1.8883x over previous
"""GQA decode attention (b=32, T=4096, 64 q-heads / 8 kv-heads) on 8 trn2 cores.

Tensor-parallel over heads: core i owns kv-head i (q-heads 8i..8i+7),
wqkv block i, KV-cache slice i, wo input-rows 1024i..1024(i+1); AllReduce
after wo.

Host-side layout prep (numerically equivalent, layout only):
  - RoPE is linear in q/k for a fixed position, so it is folded into the
    wqkv weight columns (q also absorbs the 1/sqrt(128) score scale).
  - K slice pre-transposed to [b, d, t] so score matmuls contract d on
    partitions; V packed partition-major; wo slice pre-transposed.
  - Streamed operands cast to bf16 (fp32 PSUM accumulation throughout).
"""

import math
import sys

import numpy as np

sys.path.insert(0, "/opt/trn_rl_repo")

B = 32          # batch
D = 8192        # model dim
HD = 128        # head dim
H = 8           # q-heads per core
NKV = 8         # kv heads (= cores)
T = 4096        # kv length
NT = T // 128   # t-tiles
KD = D // 128   # k-tiles over model dim
BLK = 1280      # wqkv block per kv head (8*128 q | 128 k | 128 v)

STREAM_BF16 = True   # stream K/V/weights as bf16 (fp32 accumulate)

_CACHE: dict = {}


def _build():
    from contextlib import ExitStack

    import concourse.tile as tile
    from concourse import bacc, mybir
    from concourse.masks import make_identity

    f32 = mybir.dt.float32
    dt = mybir.dt.bfloat16 if STREAM_BF16 else f32
    nc = bacc.Bacc("TRN2", target_bir_lowering=False, debug=False, num_devices=8)

    xT = nc.dram_tensor("xT", [128, KD, B], dt, kind="ExternalInput")
    wq = nc.dram_tensor("wq", [D, BLK], dt, kind="ExternalInput")
    kT = nc.dram_tensor("kT", [B, 128, T], dt, kind="ExternalInput")
    vv = nc.dram_tensor("vv", [B, 128, NT, HD], dt, kind="ExternalInput")
    woT = nc.dram_tensor("woT", [H * HD, D], dt, kind="ExternalInput")
    out_ext = nc.dram_tensor("out", [B, D], f32, kind="ExternalOutput")

    ExpF = mybir.ActivationFunctionType.Exp

    with tile.TileContext(nc) as tc, ExitStack() as ctx:
        cst = ctx.enter_context(tc.tile_pool(name="const", bufs=1))
        ident = cst.tile([128, 128], dt)
        make_identity(nc, ident[:])

        qT_sb = cst.tile([128, H, B], dt)       # q^T  [d, h, b]
        knT_sb = cst.tile([128, B], dt)         # k_new^T [d, b]
        vn_sb = cst.tile([B, HD], dt)           # v_new [b, d]
        attT_sb = cst.tile([128, H, B], dt)     # att^T [d, h, b]

        # ---------------- phase 1: fused qkv projection ----------------
        with (
            tc.tile_pool(name="w", bufs=3) as wpool,
            tc.tile_pool(name="xt", bufs=1) as xpool,
            tc.tile_pool(name="qps", bufs=1, space="PSUM") as qps,
            tc.tile_pool(name="m1", bufs=1) as m1,
            tc.tile_pool(name="tps", bufs=1, space="PSUM") as tps,
        ):
            xt = xpool.tile([128, KD, B], dt)
            nc.sync.dma_start(xt[:], xT[:])
            ps_q1 = qps.tile([B, 512], f32)
            ps_q2 = qps.tile([B, 512], f32)
            ps_kv = qps.tile([B, 256], f32)
            for k in range(KD):
                wt = wpool.tile([128, BLK], dt)
                nc.sync.dma_start(wt[:], wq[k * 128:(k + 1) * 128, :])
                lhs = xt[:, k, :]
                st, sp = k == 0, k == KD - 1
                nc.tensor.matmul(ps_q1[:], lhs, wt[:, 0:512], start=st, stop=sp)
                nc.tensor.matmul(ps_q2[:], lhs, wt[:, 512:1024], start=st, stop=sp)
                nc.tensor.matmul(ps_kv[:], lhs, wt[:, 1024:1280], start=st, stop=sp)

            q_sb = m1.tile([B, 1024], dt)
            nc.vector.tensor_copy(q_sb[:, 0:512], ps_q1[:])
            nc.vector.tensor_copy(q_sb[:, 512:1024], ps_q2[:])
            kv_sb = m1.tile([B, 256], dt)
            nc.vector.tensor_copy(kv_sb[:], ps_kv[:])
            nc.vector.tensor_copy(vn_sb[:], kv_sb[:, 128:256])

            t_ps = tps.tile([128, H, B], dt)
            for h in range(H):
                nc.tensor.transpose(
                    t_ps[:, h, :], q_sb[:, h * 128:(h + 1) * 128], ident[0:B, 0:B]
                )
            nc.vector.tensor_copy(qT_sb[:], t_ps[:])
            t2_ps = tps.tile([128, B], dt)
            nc.tensor.transpose(t2_ps[:], kv_sb[:, 0:128], ident[0:B, 0:B])
            nc.vector.tensor_copy(knT_sb[:], t2_ps[:])

        # ---------------- phase 2: attention over batches ----------------
        with (
            tc.tile_pool(name="kt", bufs=3) as ktp,
            tc.tile_pool(name="vt", bufs=3) as vtp,
            tc.tile_pool(name="pr", bufs=2) as prp,
            tc.tile_pool(name="scps", bufs=3, space="PSUM") as scp,
            tc.tile_pool(name="ovps", bufs=2, space="PSUM") as ovp,
            tc.tile_pool(name="atps", bufs=2, space="PSUM") as atp,
            tc.tile_pool(name="att", bufs=2) as attp,
        ):
            for b in range(B):
                kt_t = ktp.tile([128, T], dt)
                nc.sync.dma_start(kt_t[:], kT[b])
                # overwrite column start_pos with the new (rope'd) k
                nc.vector.tensor_copy(kt_t[:, T - 1:T], knT_sb[:, b:b + 1])

                # V tiles with a ones-column appended (softmax denominator)
                vt_t = vtp.tile([128, NT, HD + 1], dt)
                nc.sync.dma_start(vt_t[:, :, 0:HD], vv[b])
                nc.vector.memset(vt_t[:, :, HD:HD + 1], 1.0)
                # overwrite row start_pos (= tile NT-1, partition 127) with new v
                nc.sync.dma_start(vt_t[127:128, NT - 1, 0:HD], vn_sb[b:b + 1, :])

                sc = scp.tile([128, NT, H], f32)
                for j in range(NT):
                    nc.tensor.matmul(
                        sc[:, j, :], kt_t[:, j * 128:(j + 1) * 128], qT_sb[:, :, b],
                        start=True, stop=True,
                    )
                pr = prp.tile([128, NT, H], dt)
                nc.scalar.activation(pr[:], sc[:], ExpF)

                ov = ovp.tile([H, HD + 1], f32)
                for j in range(NT):
                    nc.tensor.matmul(
                        ov[:], pr[:, j, :], vt_t[:, j, :],
                        start=(j == 0), stop=(j == NT - 1),
                    )

                rec = attp.tile([H, 1], f32)
                nc.vector.reciprocal(rec[:], ov[:, HD:HD + 1])
                att_b = attp.tile([H, HD], dt)
                nc.vector.tensor_scalar_mul(att_b[:], ov[:, 0:HD], rec[:])
                at_ps = atp.tile([128, H], dt)
                nc.tensor.transpose(at_ps[:], att_b[:], ident[0:H, 0:H])
                nc.vector.tensor_copy(attT_sb[:, :, b], at_ps[:])

        # ---------------- phase 3: wo row-parallel + AllReduce ----------------
        with (
            tc.tile_pool(name="wo", bufs=3) as wop,
            tc.tile_pool(name="wops", bufs=1, space="PSUM") as wops,
            tc.tile_pool(name="ob", bufs=4) as obp,
            tc.tile_pool(name="dram", bufs=1, space="DRAM") as dram,
        ):
            cc_in = dram.tile([B, D], f32)
            cc_out = dram.tile([B, D], f32)
            for half in range(2):
                ps_list = [
                    wops.tile([B, 512], f32, name=f"wops{n}", tag=f"wops{n}")
                    for n in range(8)
                ]
                for k in range(H):
                    wt = wop.tile([128, 4096], dt)
                    nc.sync.dma_start(
                        wt[:], woT[k * 128:(k + 1) * 128, half * 4096:(half + 1) * 4096]
                    )
                    for n in range(8):
                        nc.tensor.matmul(
                            ps_list[n][:], attT_sb[:, k, :], wt[:, n * 512:(n + 1) * 512],
                            start=(k == 0), stop=(k == H - 1),
                        )
                for n in range(8):
                    ob = obp.tile([B, 512], f32)
                    nc.vector.tensor_copy(ob[:], ps_list[n][:])
                    base = half * 4096 + n * 512
                    nc.sync.dma_start(cc_in[:, base:base + 512], ob[:])

            nc.gpsimd.collective_compute(
                "AllReduce",
                mybir.AluOpType.add,
                replica_groups=[list(range(8))],
                ins=[cc_in.opt()],
                outs=[cc_out.opt()],
            )
            nc.sync.dma_start(out_ext[:], cc_out[:])

    nc.compile()
    return nc


def _prep_inputs(x, cache_k, cache_v, wqkv_w, wo_w, freqs_cos, freqs_sin):
    if STREAM_BF16:
        import ml_dtypes

        sdt = ml_dtypes.bfloat16
    else:
        sdt = np.float32
    cos = np.asarray(freqs_cos, np.float32).reshape(-1)[:64]
    sin = np.asarray(freqs_sin, np.float32).reshape(-1)[:64]
    x = np.asarray(x, np.float32).reshape(B, D)
    # x^T packed tile-major: xT[p, k, b] = x[b, 128k+p]
    xT = np.ascontiguousarray(x.reshape(B, KD, 128).transpose(2, 1, 0)).astype(sdt)

    wqkv_w = np.asarray(wqkv_w, np.float32)
    scale = 1.0 / math.sqrt(HD)
    in_maps = []
    for c in range(8):
        W = wqkv_w[:, c * BLK:(c + 1) * BLK].copy()
        q = W[:, :1024].reshape(D, H, 64, 2)
        q0 = q[..., 0].copy()
        q1 = q[..., 1].copy()
        q[..., 0] = (q0 * cos - q1 * sin) * scale
        q[..., 1] = (q0 * sin + q1 * cos) * scale
        k = W[:, 1024:1152].reshape(D, 64, 2)
        k0 = k[..., 0].copy()
        k1 = k[..., 1].copy()
        k[..., 0] = k0 * cos - k1 * sin
        k[..., 1] = k0 * sin + k1 * cos

        kTc = np.ascontiguousarray(
            np.asarray(cache_k[:, :, c, :], np.float32).transpose(0, 2, 1)
        ).astype(sdt)  # [B, 128, T]
        vc = np.ascontiguousarray(
            np.asarray(cache_v[:, :, c, :], np.float32)
            .reshape(B, NT, 128, HD)
            .transpose(0, 2, 1, 3)
        ).astype(sdt)  # [B, 128, NT, HD]  (partition-major)
        woTc = np.ascontiguousarray(
            np.asarray(wo_w[:, c * 1024:(c + 1) * 1024], np.float32).T
        ).astype(sdt)  # [1024, D]
        in_maps.append({
            "xT": xT, "wq": W.astype(sdt), "kT": kTc, "vv": vc, "woT": woTc,
        })
    return in_maps


def kernel(x, cache_k, cache_v, wqkv_w, wo_w, freqs_cos, freqs_sin, mask,
           start_pos, _want_trace=False, **_unused):
    from concourse.bass_utils import run_bass_kernel_spmd

    sp = int(np.asarray(start_pos))
    assert sp == T - 1, f"kernel compiled for start_pos={T - 1}, got {sp}"

    if "nc" not in _CACHE:
        _CACHE["nc"] = _build()
    nc = _CACHE["nc"]

    in_maps = _prep_inputs(x, cache_k, cache_v, wqkv_w, wo_w, freqs_cos, freqs_sin)
    res = run_bass_kernel_spmd(nc, in_maps, list(range(8)), trace=_want_trace)
    out = res.results[0]["out"].reshape(B, 1, D).astype(np.float32)
    if _want_trace:
        _CACHE["last_result"] = res
    return out
